# revision 32
# baseline (speedup 1.0000x reference)
"""Trainium2 Bass kernel for nn_DualAttention (8 NeuronCores).

Math: the reference's alpha/beta double-softmax collapses to a single
softmax:  gamma = softmax_s(u_d*A[s,d] + v_d*C[s,d]) with
  A = tanh(lin1(enc)) (raw-view reshaped), C = tanh(lin3(z)),
  u = tanh(lin2(out)), v = tanh(lin4(out)),
  attn[b,t,d] = sum_s gamma * enc_r[b,s,d].
The raw [S,B,E] -> [B,S,E] view means batch b's slab is
encoder_hidden.reshape(S*B, E)[b*S:(b+1)*S].

Sharding: core c -> (b = c//2, d-half h = c%2).  Each core computes
attn for its (b, 256 d-values) over all T=64 timesteps.

Two device layouts (K_LAYOUT env, default "s"):
 - "s": s on partitions. Gate ops are chunked fp16 tensor_tensor at 2x
   rate; softmax reductions over s run on the Tensor engine as
   ones-matmuls over partitions; u/v broadcast via DRAM step-0 DMA.
 - "d": d on partitions. Per-t tensor_scalar/scalar_tensor_tensor +
   affine_mul_reduce; den via ACT exp accum.
Host prep is layout/dtype only (transposes + fp16 casts + slicing).
"""

import os
import sys

sys.path.insert(0, "/opt/trn_rl_repo")

import numpy as np

B, T, S, D, E, F = 4, 64, 256, 512, 512, 128
DH = D // 2          # d per core
NBLK = 2             # 128-row blocks per core (d-blocks or s-blocks)
TCH = 8              # t chunk
N_CORES = 8

_prog_cache = {}


def _build_program_d(den_mode="act", lmode="tt"):
    import concourse.bass as bass
    import concourse.tile as tile
    from concourse import bacc, mybir

    f16 = mybir.dt.float16
    f32 = mybir.dt.float32
    MUL = mybir.AluOpType.mult
    ADD = mybir.AluOpType.add

    nc = bacc.Bacc(None, target_bir_lowering=False)

    w1t_d = nc.dram_tensor("w1t", [128, 4, DH], f16, kind="ExternalInput")
    w2t_d = nc.dram_tensor("w2t", [128, 4, DH], f16, kind="ExternalInput")
    w4t_d = nc.dram_tensor("w4t", [128, 4, DH], f16, kind="ExternalInput")
    w3t_d = nc.dram_tensor("w3t", [128, DH], f16, kind="ExternalInput")
    encmm_d = nc.dram_tensor("encmm", [128, 4, S], f16, kind="ExternalInput")
    encg_d = nc.dram_tensor("encg", [128, NBLK, S], f16, kind="ExternalInput")
    outmm_d = nc.dram_tensor("outmm", [128, 4, T], f16, kind="ExternalInput")
    zmm_d = nc.dram_tensor("zmm", [128, S], f16, kind="ExternalInput")
    b1_d = nc.dram_tensor("b1", [128, NBLK], f32, kind="ExternalInput")
    b2_d = nc.dram_tensor("b2", [128, NBLK], f32, kind="ExternalInput")
    b3_d = nc.dram_tensor("b3", [128, NBLK], f32, kind="ExternalInput")
    b4_d = nc.dram_tensor("b4", [128, NBLK], f32, kind="ExternalInput")
    attn_d = nc.dram_tensor("attn_t", [128, NBLK, T], f32, kind="ExternalOutput")

    with tile.TileContext(nc) as tc:
        with (
            tc.tile_pool(name="weights", bufs=1) as wpool,
            tc.tile_pool(name="acts", bufs=1) as apool,
            tc.tile_pool(name="gate", bufs=3) as gpool,
            tc.tile_pool(name="scr", bufs=4) as spool,
            tc.tile_pool(name="psum", bufs=2, space="PSUM") as psum,
            tc.tile_pool(name="psum_uv", bufs=2, space="PSUM") as psum_uv,
        ):
            w1t = wpool.tile([128, 4, DH], f16)
            w2t = wpool.tile([128, 4, DH], f16)
            w4t = wpool.tile([128, 4, DH], f16)
            w3t = wpool.tile([128, DH], f16)
            encmm = wpool.tile([128, 4, S], f16)
            encg = wpool.tile([128, NBLK, S], f16)
            outmm = wpool.tile([128, 4, T], f16)
            zmm = wpool.tile([128, S], f16)
            b1s = wpool.tile([128, NBLK], f32)
            b2s = wpool.tile([128, NBLK], f32)
            b3s = wpool.tile([128, NBLK], f32)
            b4s = wpool.tile([128, NBLK], f32)
            for t, d in [(w1t, w1t_d), (w2t, w2t_d), (w4t, w4t_d), (w3t, w3t_d),
                         (encmm, encmm_d), (encg, encg_d), (outmm, outmm_d),
                         (zmm, zmm_d), (b1s, b1_d), (b2s, b2_d), (b3s, b3_d),
                         (b4s, b4_d)]:
                nc.sync.dma_start(t[:], d[:])

            A = apool.tile([128, NBLK, S], f16)
            C = apool.tile([128, NBLK, S], f16)
            U = apool.tile([128, NBLK, T], f32)
            V = apool.tile([128, NBLK, T], f32)
            Tanh = mybir.ActivationFunctionType.Tanh

            for blk in range(NBLK):
                dcols = bass.ts(blk, 128)
                pa = psum.tile([128, S], f32)
                for k in range(4):
                    nc.tensor.matmul(pa[:], w1t[:, k, dcols], encmm[:, k, :],
                                     start=(k == 0), stop=(k == 3))
                nc.scalar.activation(out=A[:, blk, :], in_=pa[:], func=Tanh,
                                     bias=b1s[:, blk:blk + 1], scale=1.0)

                pc = psum.tile([128, S], f32)
                nc.tensor.matmul(pc[:], w3t[:, dcols], zmm[:], start=True, stop=True)
                nc.scalar.activation(out=C[:, blk, :], in_=pc[:], func=Tanh,
                                     bias=b3s[:, blk:blk + 1], scale=1.0)

                pu = psum_uv.tile([128, T], f32)
                for k in range(4):
                    nc.tensor.matmul(pu[:], w2t[:, k, dcols], outmm[:, k, :],
                                     start=(k == 0), stop=(k == 3))
                nc.scalar.activation(out=U[:, blk, :], in_=pu[:], func=Tanh,
                                     bias=b2s[:, blk:blk + 1], scale=1.0)

                pv = psum_uv.tile([128, T], f32)
                for k in range(4):
                    nc.tensor.matmul(pv[:], w4t[:, k, dcols], outmm[:, k, :],
                                     start=(k == 0), stop=(k == 3))
                nc.scalar.activation(out=V[:, blk, :], in_=pv[:], func=Tanh,
                                     bias=b4s[:, blk:blk + 1], scale=1.0)

            num = apool.tile([128, NBLK, T], f32, tag="num")
            den = apool.tile([128, NBLK, T], f32, tag="den")
            for blk in range(NBLK):
                for tc_i in range(T // TCH):
                    m2c = gpool.tile([128, TCH, S], f16, tag="m2c")
                    Lc = gpool.tile([128, TCH, S], f16, tag="Lc")
                    Ec = gpool.tile([128, TCH, S], f16, tag="Ec")
                    m1c = None
                    if lmode == "tt":
                        m1c = gpool.tile([128, TCH, S], f16, tag="m1c")
                    for j in range(TCH):
                        t = tc_i * TCH + j
                        nc.vector.tensor_scalar(
                            out=m2c[:, j, :], in0=C[:, blk, :],
                            scalar1=V[:, blk, t:t + 1], scalar2=None, op0=MUL)
                        if lmode == "tt":
                            nc.vector.tensor_scalar(
                                out=m1c[:, j, :], in0=A[:, blk, :],
                                scalar1=U[:, blk, t:t + 1], scalar2=None, op0=MUL)
                        else:
                            nc.vector.scalar_tensor_tensor(
                                out=Lc[:, j, :], in0=A[:, blk, :],
                                scalar=U[:, blk, t:t + 1], in1=m2c[:, j, :],
                                op0=MUL, op1=ADD)
                    if lmode == "tt":
                        nc.vector.tensor_tensor(out=Lc[:], in0=m1c[:], in1=m2c[:],
                                                op=ADD)
                    if den_mode == "act":
                        for j in range(TCH):
                            t = tc_i * TCH + j
                            nc.scalar.activation(
                                out=Ec[:, j, :], in_=Lc[:, j, :],
                                func=mybir.ActivationFunctionType.Exp,
                                accum_out=den[:, blk, t:t + 1])
                    else:
                        nc.scalar.activation(out=Ec[:], in_=Lc[:],
                                             func=mybir.ActivationFunctionType.Exp)
                        nc.vector.tensor_reduce(
                            out=den[:, blk, tc_i * TCH:(tc_i + 1) * TCH],
                            in_=Ec[:], axis=mybir.AxisListType.X, op=ADD)
                    for j in range(TCH):
                        t = tc_i * TCH + j
                        pscr = spool.tile([128, S], f16, tag="pscr")
                        nc.vector.affine_mul_reduce(
                            out=pscr[:], accum_out=num[:, blk, t:t + 1],
                            in0=Ec[:, j, :], in1=encg[:, blk, :], scale=1.0,
                            bias=0.0)

                rden = spool.tile([128, T], f32, tag="rden")
                attn_t = spool.tile([128, T], f32, tag="attn")
                nc.vector.reciprocal(out=rden[:], in_=den[:, blk, :])
                nc.vector.tensor_tensor(out=attn_t[:], in0=num[:, blk, :],
                                        in1=rden[:], op=MUL)
                nc.sync.dma_start(attn_d[:, blk, :], attn_t[:])

    nc.finalize()
    return nc


def _build_program_s():
    """s-on-partitions layout."""
    import concourse.bass as bass
    import concourse.tile as tile
    from concourse import bacc, mybir

    f16 = mybir.dt.float16
    f32 = mybir.dt.float32
    MUL = mybir.AluOpType.mult
    ADD = mybir.AluOpType.add
    Tanh = mybir.ActivationFunctionType.Tanh
    NTC = T // TCH          # 8 t-chunks
    CH = TCH * DH           # 2048 free per chunk

    nc = bacc.Bacc(None, target_bir_lowering=False)

    w1t_d = nc.dram_tensor("w1t", [128, 4, DH], f16, kind="ExternalInput")
    w2t_d = nc.dram_tensor("w2t", [128, 4, DH], f16, kind="ExternalInput")
    w4t_d = nc.dram_tensor("w4t", [128, 4, DH], f16, kind="ExternalInput")
    w3t_d = nc.dram_tensor("w3t", [128, DH], f16, kind="ExternalInput")
    w1b_d = nc.dram_tensor("w1b", [1, DH], f16, kind="ExternalInput")
    w2b_d = nc.dram_tensor("w2b", [1, DH], f16, kind="ExternalInput")
    w3b_d = nc.dram_tensor("w3b", [1, DH], f16, kind="ExternalInput")
    w4b_d = nc.dram_tensor("w4b", [1, DH], f16, kind="ExternalInput")
    encmm_d = nc.dram_tensor("encmm", [128, 4, S], f16, kind="ExternalInput")
    outmm_d = nc.dram_tensor("outmm", [128, 4, T], f16, kind="ExternalInput")
    zmm_d = nc.dram_tensor("zmm", [128, S], f16, kind="ExternalInput")
    encrep_d = nc.dram_tensor("encrep", [128, NBLK, CH], f16, kind="ExternalInput")
    attn_d = nc.dram_tensor("attn_s", [T, DH], f32, kind="ExternalOutput")

    with tile.TileContext(nc) as tc:
        with (
            tc.tile_pool(name="weights", bufs=1) as wpool,
            tc.tile_pool(name="acts", bufs=1) as apool,
            tc.tile_pool(name="bc", bufs=6) as bcpool,
            tc.tile_pool(name="gate", bufs=4) as gpool,
            tc.tile_pool(name="scr", bufs=2) as spool,
            tc.tile_pool(name="dram", bufs=2, space="DRAM") as dpool,
        ):
            w1t = wpool.tile([128, 4, DH], f16)
            w2t = wpool.tile([128, 4, DH], f16)
            w4t = wpool.tile([128, 4, DH], f16)
            w3t = wpool.tile([128, DH], f16)
            w1b = wpool.tile([1, DH], f16)
            w2b = wpool.tile([1, DH], f16)
            w3b = wpool.tile([1, DH], f16)
            w4b = wpool.tile([1, DH], f16)
            encmm = wpool.tile([128, 4, S], f16)
            outmm = wpool.tile([128, 4, T], f16)
            zmm = wpool.tile([128, S], f16)
            encrep = wpool.tile([128, NBLK, CH], f16)
            for t, d in [(outmm, outmm_d), (w2t, w2t_d), (w2b, w2b_d)]:
                nc.sync.dma_start(t[:], d[:])
            for t, d in [(w4t, w4t_d), (w4b, w4b_d)]:
                nc.scalar.dma_start(t[:], d[:])
            for t, d in [(encmm, encmm_d), (w1t, w1t_d), (w3t, w3t_d),
                         (w1b, w1b_d), (w3b, w3b_d), (zmm, zmm_d),
                         (encrep, encrep_d)]:
                nc.scalar.dma_start(t[:], d[:])

            ones = wpool.tile([128, 128], f16)
            nc.vector.memset(ones[:], 1.0)

            # ---- linears: U/V first (they feed the per-tc bcast DMAs) ----
            psum_ctx = tc.tile_pool(name="psum", bufs=2, space="PSUM")
            psum = psum_ctx.__enter__()
            A_nat = apool.tile([128, NBLK, DH], f16)
            C_nat = apool.tile([128, NBLK, DH], f16)
            A_rep = apool.tile([128, NBLK, CH], f16)
            C_rep = apool.tile([128, NBLK, CH], f16)
            U_td = apool.tile([T, DH], f16)
            V_td = apool.tile([T, DH], f16)

            pu = psum.tile([T, DH], f32)
            for k in range(4):
                nc.tensor.matmul(pu[:], outmm[:, k, :], w2t[:, k, :],
                                 start=(k == 0), stop=False)
            nc.tensor.matmul(pu[:], ones[0:1, 0:T], w2b[:], start=False, stop=True)
            nc.scalar.activation(out=U_td[:], in_=pu[:], func=Tanh)

            pv = psum.tile([T, DH], f32)
            for k in range(4):
                nc.tensor.matmul(pv[:], outmm[:, k, :], w4t[:, k, :],
                                 start=(k == 0), stop=False)
            nc.tensor.matmul(pv[:], ones[0:1, 0:T], w4b[:], start=False, stop=True)
            nc.scalar.activation(out=V_td[:], in_=pv[:], func=Tanh)

            dram_u = dpool.tile([T, DH], f16)
            dram_v = dpool.tile([T, DH], f16)
            nc.sync.dma_start(dram_u[:], U_td[:])
            nc.sync.dma_start(dram_v[:], V_td[:])

            for sblk in range(NBLK):
                scols = bass.ts(sblk, 128)
                pa = psum.tile([128, DH], f32)
                for k in range(4):
                    nc.tensor.matmul(pa[:], encmm[:, k, scols], w1t[:, k, :],
                                     start=(k == 0), stop=False)
                nc.tensor.matmul(pa[:], ones[0:1, 0:128], w1b[:],
                                 start=False, stop=True)
                nc.scalar.activation(out=A_nat[:, sblk, :], in_=pa[:], func=Tanh)

                pc = psum.tile([128, DH], f32)
                nc.tensor.matmul(pc[:], zmm[:, scols], w3t[:], start=True,
                                 stop=False)
                nc.tensor.matmul(pc[:], ones[0:1, 0:128], w3b[:],
                                 start=False, stop=True)
                nc.scalar.activation(out=C_nat[:, sblk, :], in_=pc[:], func=Tanh)
                # replicate this sblk's A/C right away (step-0 free-dim read)
                for rep_dst, nat_src in [(A_rep, A_nat), (C_rep, C_nat)]:
                    src_ap = bass.AP(
                        tensor=nat_src.tensor,
                        offset=nat_src.offset + sblk * DH,
                        ap=[nat_src.ap[0], [0, TCH], [1, DH]],
                    )
                    nc.scalar.dma_start(rep_dst[:, sblk, :].rearrange(
                        "p (tch dh) -> p tch dh", tch=TCH), src_ap)
            psum_ctx.__exit__(None, None, None)
            psum_r_ctx = tc.tile_pool(name="psum_r", bufs=2, space="PSUM")
            psum_r = psum_r_ctx.__enter__()

            # ---- gate loop ----
            numsc = spool.tile([T, DH], f32, tag="numsc")
            densc = spool.tile([T, DH], f32, tag="densc")
            def flush(pend):
                # P-mult + PE reduces for a completed (tc, sblk); stage is
                # software-pipelined one iteration behind the exp producer.
                Ec_p, tcp, sbp, pnd_p = pend
                Pc = gpool.tile([128, CH], f16, tag="Pc")
                nc.vector.tensor_tensor(out=Pc[:], in0=Ec_p[:],
                                        in1=encrep[:, sbp, :], op=MUL)
                for sl in range(CH // 512):
                    cs = bass.ts(sl, 512)
                    nc.tensor.matmul(pnd_p[0:1, cs], ones[:, 0:1], Ec_p[:, cs],
                                     start=(sbp == 0), stop=(sbp == 1))
                    nc.tensor.matmul(pnd_p[32:33, cs], ones[:, 0:1], Pc[:, cs],
                                     start=(sbp == 0), stop=(sbp == 1))
                if sbp == NBLK - 1:
                    ndrow = spool.tile([33, CH], f32, tag="ndrow")
                    nc.scalar.copy(ndrow[:], pnd_p[0:33, :])
                    trows = slice(tcp * TCH, (tcp + 1) * TCH)
                    nc.sync.dma_start(
                        numsc[trows, :],
                        ndrow[32:33, :].rearrange("p (t d) -> p t d", t=TCH))
                    nc.sync.dma_start(
                        densc[trows, :],
                        ndrow[0:1, :].rearrange("p (t d) -> p t d", t=TCH))

            pend = None
            for tc_i in range(NTC):
                u_bc = bcpool.tile([128, CH], f16, tag="u_bc")
                v_bc = bcpool.tile([128, CH], f16, tag="v_bc")
                usrc = bass.AP(tensor=dram_u.tensor,
                               offset=dram_u.offset + tc_i * CH,
                               ap=[[0, 128], [1, CH]])
                vsrc = bass.AP(tensor=dram_v.tensor,
                               offset=dram_v.offset + tc_i * CH,
                               ap=[[0, 128], [1, CH]])
                nc.sync.dma_start(u_bc[:], usrc)
                nc.sync.dma_start(v_bc[:], vsrc)

                # den -> PSUM partition 0, num -> partition 32 of ONE
                # 4-bank tile: single ACT copy reads both; 2 bufs pipeline.
                # Both sblocks interleaved so no DVE op reads the output of
                # the immediately-preceding DVE op (drain serialization).
                pnd = psum_r.tile([64, CH], f32, tag="pnd")
                m1a = gpool.tile([128, CH], f16, tag="m1")
                m2a = gpool.tile([128, CH], f16, tag="m2")
                Eca = gpool.tile([128, CH], f16, tag="Ec")
                m1b = gpool.tile([128, CH], f16, tag="m1")
                m2b = gpool.tile([128, CH], f16, tag="m2")
                Ecb = gpool.tile([128, CH], f16, tag="Ec")
                nc.vector.tensor_tensor(out=m1a[:], in0=A_rep[:, 0, :],
                                        in1=u_bc[:], op=MUL)
                nc.vector.tensor_tensor(out=m2a[:], in0=C_rep[:, 0, :],
                                        in1=v_bc[:], op=MUL)
                nc.vector.tensor_tensor(out=m1b[:], in0=A_rep[:, 1, :],
                                        in1=u_bc[:], op=MUL)
                nc.vector.tensor_tensor(out=m2b[:], in0=C_rep[:, 1, :],
                                        in1=v_bc[:], op=MUL)
                nc.vector.tensor_tensor(out=m1a[:], in0=m1a[:], in1=m2a[:],
                                        op=ADD)
                nc.vector.tensor_tensor(out=m1b[:], in0=m1b[:], in1=m2b[:],
                                        op=ADD)
                nc.scalar.activation(out=Eca[:], in_=m1a[:],
                                     func=mybir.ActivationFunctionType.Exp)
                nc.scalar.activation(out=Ecb[:], in_=m1b[:],
                                     func=mybir.ActivationFunctionType.Exp)
                if pend is not None:
                    flush(pend[0])
                    flush(pend[1])
                pend = ((Eca, tc_i, 0, pnd), (Ecb, tc_i, 1, pnd))
            flush(pend[0])
            flush(pend[1])

            TH = T // 2
            rden = spool.tile([T, DH], f32, tag="rden")
            attn_s = spool.tile([T, DH], f32, tag="attn_s")
            for half in range(2):
                hr = slice(half * TH, (half + 1) * TH)
                nc.vector.reciprocal(out=rden[hr, :], in_=densc[hr, :])
                nc.vector.tensor_tensor(out=attn_s[hr, :], in0=numsc[hr, :],
                                        in1=rden[hr, :], op=MUL)
                nc.sync.dma_start(attn_d[hr, :], attn_s[hr, :])
            psum_r_ctx.__exit__(None, None, None)

    nc.finalize()
    return nc


def _get_program():
    if os.environ.get("K_LAYOUT", "s") == "s":
        if "s" not in _prog_cache:
            _prog_cache["s"] = _build_program_s()
        return _prog_cache["s"]
    key = (os.environ.get("K_DEN", "act"), os.environ.get("K_L", "tt"))
    if key not in _prog_cache:
        _prog_cache[key] = _build_program_d(den_mode=key[0], lmode=key[1])
    return _prog_cache[key]


def _host_prep(output, encoder_hidden, input_z, W1, b1, W2, b2, W3, b3, W4, b4):
    """Layout-only transforms -> per-core input maps."""
    f16 = np.float16
    H = np.ascontiguousarray(encoder_hidden).reshape(S * B, E)
    W1T = W1.T.astype(f16)   # [E, D]
    W2T = W2.T.astype(f16)
    W4T = W4.T.astype(f16)
    W3T = W3.T.astype(f16)   # [F, D]
    slayout = os.environ.get("K_LAYOUT", "s") == "s"
    in_maps = []
    for c in range(N_CORES):
        b, h = divmod(c, 2)
        dcols = slice(h * DH, (h + 1) * DH)
        slab = H[b * S:(b + 1) * S]                       # [S, E]
        encmm = np.ascontiguousarray(slab.T).astype(f16)  # [E, S]

        def kmajor(x, k):          # [k*128, X] -> [128, k, X]
            return np.ascontiguousarray(x.reshape(k, 128, -1).transpose(1, 0, 2))

        if slayout:
            encs = slab[:, dcols].astype(f16)              # [S, DH]
            encrep = np.ascontiguousarray(
                np.tile(encs.reshape(2, 128, 1, DH), (1, 1, TCH, 1))
                .transpose(1, 0, 2, 3)                     # [128, NBLK, TCH, DH]
            ).reshape(128, NBLK, TCH * DH)
            in_maps.append({
                "w1t": kmajor(W1T[:, dcols], 4),
                "w2t": kmajor(W2T[:, dcols], 4),
                "w4t": kmajor(W4T[:, dcols], 4),
                "w3t": np.ascontiguousarray(W3T[:, dcols]),
                "w1b": b1[dcols].reshape(1, DH).astype(f16),
                "w2b": b2[dcols].reshape(1, DH).astype(f16),
                "w3b": b3[dcols].reshape(1, DH).astype(f16),
                "w4b": b4[dcols].reshape(1, DH).astype(f16),
                "encmm": kmajor(encmm, 4),
                "outmm": kmajor(np.ascontiguousarray(output[b].T).astype(f16), 4),
                "zmm": np.ascontiguousarray(input_z[b].T).astype(f16),
                "encrep": encrep,
            })
            continue
        in_maps.append({
            "w1t": kmajor(W1T[:, dcols], 4),
            "w2t": kmajor(W2T[:, dcols], 4),
            "w4t": kmajor(W4T[:, dcols], 4),
            "w3t": np.ascontiguousarray(W3T[:, dcols]),
            "encmm": kmajor(encmm, 4),
            "encg": kmajor(encmm[h * DH:(h + 1) * DH], NBLK),
            "outmm": kmajor(np.ascontiguousarray(output[b].T).astype(f16), 4),
            "zmm": np.ascontiguousarray(input_z[b].T).astype(f16),
            "b1": np.ascontiguousarray(b1[dcols].reshape(NBLK, 128).T).astype(np.float32),
            "b2": np.ascontiguousarray(b2[dcols].reshape(NBLK, 128).T).astype(np.float32),
            "b3": np.ascontiguousarray(b3[dcols].reshape(NBLK, 128).T).astype(np.float32),
            "b4": np.ascontiguousarray(b4[dcols].reshape(NBLK, 128).T).astype(np.float32),
        })
    return in_maps


def _install_ntff_hook():
    """The agent image's antenv lacks axon_hooks; synthesize it so
    run_bass_kernel_spmd(trace=True) can collect NTFF profiles."""
    import types

    if "antenv.axon_hooks" in sys.modules:
        return
    import antenv

    mod = types.ModuleType("antenv.axon_hooks")
    holder = {"h": None}
    mod.set_axon_ntff_profile_hook = lambda h: holder.__setitem__("h", h)
    mod.get_axon_ntff_profile_hook = lambda: holder["h"]
    sys.modules["antenv.axon_hooks"] = mod
    antenv.axon_hooks = mod
    try:
        from trn_agent_boot.trn_boot import _ntff_profile_via_ctypes

        h = _ntff_profile_via_ctypes("/opt/axon/libaxon_pjrt.so")
        if h is not None:
            mod.set_axon_ntff_profile_hook(h)
    except Exception as e:
        print(f"ntff hook install failed: {e}", file=sys.stderr)


def _run(inputs, trace=False):
    from concourse.bass_utils import run_bass_kernel_spmd

    if trace:
        try:
            _install_ntff_hook()
        except Exception as e:
            print(f"ntff hook unavailable: {e}", file=sys.stderr)

    nc = _get_program()
    in_maps = _host_prep(**inputs)
    res = run_bass_kernel_spmd(
        nc, in_maps, core_ids=list(range(N_CORES)), trace=trace,
    )
    output = np.asarray(inputs["output"], dtype=np.float32)
    attn = np.empty((B, T, D), np.float32)
    slayout = os.environ.get("K_LAYOUT", "s") == "s"
    for c in range(N_CORES):
        b, h = divmod(c, 2)
        if slayout:
            attn[b, :, h * DH:(h + 1) * DH] = res.results[c]["attn_s"]
        else:
            at = res.results[c]["attn_t"]                 # [128, NBLK, T]
            at = at.transpose(1, 0, 2).reshape(DH, T)     # [d, t]
            attn[b, :, h * DH:(h + 1) * DH] = at.T
    concat = np.concatenate([output, attn], axis=-1)
    return (concat, attn), res


def kernel(**inputs):
    inputs = {k: np.asarray(v) for k, v in inputs.items()}
    (concat, attn), _ = _run(inputs, trace=False)
    return concat, attn


# revision 33
# speedup vs baseline: 1.0713x; 1.0713x over previous
"""Trainium2 Bass kernel for nn_DualAttention (8 NeuronCores).

Math: the reference's alpha/beta double-softmax collapses to a single
softmax:  gamma = softmax_s(u_d*A[s,d] + v_d*C[s,d]) with
  A = tanh(lin1(enc)) (raw-view reshaped), C = tanh(lin3(z)),
  u = tanh(lin2(out)), v = tanh(lin4(out)),
  attn[b,t,d] = sum_s gamma * enc_r[b,s,d].
The raw [S,B,E] -> [B,S,E] view means batch b's slab is
encoder_hidden.reshape(S*B, E)[b*S:(b+1)*S].

Sharding: core c -> (b = c//2, d-half h = c%2).  Each core computes
attn for its (b, 256 d-values) over all T=64 timesteps.

Two device layouts (K_LAYOUT env, default "s"):
 - "s": s on partitions. Gate ops are chunked fp16 tensor_tensor at 2x
   rate; softmax reductions over s run on the Tensor engine as
   ones-matmuls over partitions; u/v broadcast via DRAM step-0 DMA.
 - "d": d on partitions. Per-t tensor_scalar/scalar_tensor_tensor +
   affine_mul_reduce; den via ACT exp accum.
Host prep is layout/dtype only (transposes + fp16 casts + slicing).
"""

import os
import sys

sys.path.insert(0, "/opt/trn_rl_repo")

import numpy as np

B, T, S, D, E, F = 4, 64, 256, 512, 512, 128
DH = D // 2          # d per core
NBLK = 2             # 128-row blocks per core (d-blocks or s-blocks)
TCH = 8              # t chunk
N_CORES = 8

_prog_cache = {}


def _build_program_d(den_mode="act", lmode="tt"):
    import concourse.bass as bass
    import concourse.tile as tile
    from concourse import bacc, mybir

    f16 = mybir.dt.float16
    f32 = mybir.dt.float32
    MUL = mybir.AluOpType.mult
    ADD = mybir.AluOpType.add

    nc = bacc.Bacc(None, target_bir_lowering=False)

    w1t_d = nc.dram_tensor("w1t", [128, 4, DH], f16, kind="ExternalInput")
    w2t_d = nc.dram_tensor("w2t", [128, 4, DH], f16, kind="ExternalInput")
    w4t_d = nc.dram_tensor("w4t", [128, 4, DH], f16, kind="ExternalInput")
    w3t_d = nc.dram_tensor("w3t", [128, DH], f16, kind="ExternalInput")
    encmm_d = nc.dram_tensor("encmm", [128, 4, S], f16, kind="ExternalInput")
    encg_d = nc.dram_tensor("encg", [128, NBLK, S], f16, kind="ExternalInput")
    outmm_d = nc.dram_tensor("outmm", [128, 4, T], f16, kind="ExternalInput")
    zmm_d = nc.dram_tensor("zmm", [128, S], f16, kind="ExternalInput")
    b1_d = nc.dram_tensor("b1", [128, NBLK], f32, kind="ExternalInput")
    b2_d = nc.dram_tensor("b2", [128, NBLK], f32, kind="ExternalInput")
    b3_d = nc.dram_tensor("b3", [128, NBLK], f32, kind="ExternalInput")
    b4_d = nc.dram_tensor("b4", [128, NBLK], f32, kind="ExternalInput")
    attn_d = nc.dram_tensor("attn_t", [128, NBLK, T], f32, kind="ExternalOutput")

    with tile.TileContext(nc) as tc:
        with (
            tc.tile_pool(name="weights", bufs=1) as wpool,
            tc.tile_pool(name="acts", bufs=1) as apool,
            tc.tile_pool(name="gate", bufs=3) as gpool,
            tc.tile_pool(name="scr", bufs=4) as spool,
            tc.tile_pool(name="psum", bufs=2, space="PSUM") as psum,
            tc.tile_pool(name="psum_uv", bufs=2, space="PSUM") as psum_uv,
        ):
            w1t = wpool.tile([128, 4, DH], f16)
            w2t = wpool.tile([128, 4, DH], f16)
            w4t = wpool.tile([128, 4, DH], f16)
            w3t = wpool.tile([128, DH], f16)
            encmm = wpool.tile([128, 4, S], f16)
            encg = wpool.tile([128, NBLK, S], f16)
            outmm = wpool.tile([128, 4, T], f16)
            zmm = wpool.tile([128, S], f16)
            b1s = wpool.tile([128, NBLK], f32)
            b2s = wpool.tile([128, NBLK], f32)
            b3s = wpool.tile([128, NBLK], f32)
            b4s = wpool.tile([128, NBLK], f32)
            for t, d in [(w1t, w1t_d), (w2t, w2t_d), (w4t, w4t_d), (w3t, w3t_d),
                         (encmm, encmm_d), (encg, encg_d), (outmm, outmm_d),
                         (zmm, zmm_d), (b1s, b1_d), (b2s, b2_d), (b3s, b3_d),
                         (b4s, b4_d)]:
                nc.sync.dma_start(t[:], d[:])

            A = apool.tile([128, NBLK, S], f16)
            C = apool.tile([128, NBLK, S], f16)
            U = apool.tile([128, NBLK, T], f32)
            V = apool.tile([128, NBLK, T], f32)
            Tanh = mybir.ActivationFunctionType.Tanh

            for blk in range(NBLK):
                dcols = bass.ts(blk, 128)
                pa = psum.tile([128, S], f32)
                for k in range(4):
                    nc.tensor.matmul(pa[:], w1t[:, k, dcols], encmm[:, k, :],
                                     start=(k == 0), stop=(k == 3))
                nc.scalar.activation(out=A[:, blk, :], in_=pa[:], func=Tanh,
                                     bias=b1s[:, blk:blk + 1], scale=1.0)

                pc = psum.tile([128, S], f32)
                nc.tensor.matmul(pc[:], w3t[:, dcols], zmm[:], start=True, stop=True)
                nc.scalar.activation(out=C[:, blk, :], in_=pc[:], func=Tanh,
                                     bias=b3s[:, blk:blk + 1], scale=1.0)

                pu = psum_uv.tile([128, T], f32)
                for k in range(4):
                    nc.tensor.matmul(pu[:], w2t[:, k, dcols], outmm[:, k, :],
                                     start=(k == 0), stop=(k == 3))
                nc.scalar.activation(out=U[:, blk, :], in_=pu[:], func=Tanh,
                                     bias=b2s[:, blk:blk + 1], scale=1.0)

                pv = psum_uv.tile([128, T], f32)
                for k in range(4):
                    nc.tensor.matmul(pv[:], w4t[:, k, dcols], outmm[:, k, :],
                                     start=(k == 0), stop=(k == 3))
                nc.scalar.activation(out=V[:, blk, :], in_=pv[:], func=Tanh,
                                     bias=b4s[:, blk:blk + 1], scale=1.0)

            num = apool.tile([128, NBLK, T], f32, tag="num")
            den = apool.tile([128, NBLK, T], f32, tag="den")
            for blk in range(NBLK):
                for tc_i in range(T // TCH):
                    m2c = gpool.tile([128, TCH, S], f16, tag="m2c")
                    Lc = gpool.tile([128, TCH, S], f16, tag="Lc")
                    Ec = gpool.tile([128, TCH, S], f16, tag="Ec")
                    m1c = None
                    if lmode == "tt":
                        m1c = gpool.tile([128, TCH, S], f16, tag="m1c")
                    for j in range(TCH):
                        t = tc_i * TCH + j
                        nc.vector.tensor_scalar(
                            out=m2c[:, j, :], in0=C[:, blk, :],
                            scalar1=V[:, blk, t:t + 1], scalar2=None, op0=MUL)
                        if lmode == "tt":
                            nc.vector.tensor_scalar(
                                out=m1c[:, j, :], in0=A[:, blk, :],
                                scalar1=U[:, blk, t:t + 1], scalar2=None, op0=MUL)
                        else:
                            nc.vector.scalar_tensor_tensor(
                                out=Lc[:, j, :], in0=A[:, blk, :],
                                scalar=U[:, blk, t:t + 1], in1=m2c[:, j, :],
                                op0=MUL, op1=ADD)
                    if lmode == "tt":
                        nc.vector.tensor_tensor(out=Lc[:], in0=m1c[:], in1=m2c[:],
                                                op=ADD)
                    if den_mode == "act":
                        for j in range(TCH):
                            t = tc_i * TCH + j
                            nc.scalar.activation(
                                out=Ec[:, j, :], in_=Lc[:, j, :],
                                func=mybir.ActivationFunctionType.Exp,
                                accum_out=den[:, blk, t:t + 1])
                    else:
                        nc.scalar.activation(out=Ec[:], in_=Lc[:],
                                             func=mybir.ActivationFunctionType.Exp)
                        nc.vector.tensor_reduce(
                            out=den[:, blk, tc_i * TCH:(tc_i + 1) * TCH],
                            in_=Ec[:], axis=mybir.AxisListType.X, op=ADD)
                    for j in range(TCH):
                        t = tc_i * TCH + j
                        pscr = spool.tile([128, S], f16, tag="pscr")
                        nc.vector.affine_mul_reduce(
                            out=pscr[:], accum_out=num[:, blk, t:t + 1],
                            in0=Ec[:, j, :], in1=encg[:, blk, :], scale=1.0,
                            bias=0.0)

                rden = spool.tile([128, T], f32, tag="rden")
                attn_t = spool.tile([128, T], f32, tag="attn")
                nc.vector.reciprocal(out=rden[:], in_=den[:, blk, :])
                nc.vector.tensor_tensor(out=attn_t[:], in0=num[:, blk, :],
                                        in1=rden[:], op=MUL)
                nc.sync.dma_start(attn_d[:, blk, :], attn_t[:])

    nc.finalize()
    return nc


def _build_program_s():
    """s-on-partitions layout."""
    import concourse.bass as bass
    import concourse.tile as tile
    from concourse import bacc, mybir

    f16 = mybir.dt.float16
    f32 = mybir.dt.float32
    MUL = mybir.AluOpType.mult
    ADD = mybir.AluOpType.add
    Tanh = mybir.ActivationFunctionType.Tanh
    NTC = T // TCH          # 8 t-chunks
    CH = TCH * DH           # 2048 free per chunk

    nc = bacc.Bacc(None, target_bir_lowering=False)

    w1t_d = nc.dram_tensor("w1t", [128, 4, DH], f16, kind="ExternalInput")
    w2t_d = nc.dram_tensor("w2t", [128, 4, DH], f16, kind="ExternalInput")
    w4t_d = nc.dram_tensor("w4t", [128, 4, DH], f16, kind="ExternalInput")
    w3t_d = nc.dram_tensor("w3t", [128, DH], f16, kind="ExternalInput")
    w1b_d = nc.dram_tensor("w1b", [1, DH], f16, kind="ExternalInput")
    w2b_d = nc.dram_tensor("w2b", [1, DH], f16, kind="ExternalInput")
    w3b_d = nc.dram_tensor("w3b", [1, DH], f16, kind="ExternalInput")
    w4b_d = nc.dram_tensor("w4b", [1, DH], f16, kind="ExternalInput")
    encmm_d = nc.dram_tensor("encmm", [128, 4, S], f16, kind="ExternalInput")
    outmm_d = nc.dram_tensor("outmm", [128, 4, T], f16, kind="ExternalInput")
    zmm_d = nc.dram_tensor("zmm", [128, S], f16, kind="ExternalInput")
    encrep_d = nc.dram_tensor("encrep", [128, NBLK, CH], f16, kind="ExternalInput")
    attn_d = nc.dram_tensor("attn_s", [T, DH], f32, kind="ExternalOutput")

    with tile.TileContext(nc) as tc:
        with (
            tc.tile_pool(name="weights", bufs=1) as wpool,
            tc.tile_pool(name="acts", bufs=1) as apool,
            tc.tile_pool(name="bc", bufs=6) as bcpool,
            tc.tile_pool(name="gate", bufs=4) as gpool,
            tc.tile_pool(name="scr", bufs=2) as spool,
            tc.tile_pool(name="dram", bufs=2, space="DRAM") as dpool,
        ):
            w1t = wpool.tile([128, 4, DH], f16)
            w2t = wpool.tile([128, 4, DH], f16)
            w4t = wpool.tile([128, 4, DH], f16)
            w3t = wpool.tile([128, DH], f16)
            w1b = wpool.tile([1, DH], f16)
            w2b = wpool.tile([1, DH], f16)
            w3b = wpool.tile([1, DH], f16)
            w4b = wpool.tile([1, DH], f16)
            encmm = wpool.tile([128, 4, S], f16)
            outmm = wpool.tile([128, 4, T], f16)
            zmm = wpool.tile([128, S], f16)
            encrep = wpool.tile([128, NBLK, CH], f16)
            for t, d in [(outmm, outmm_d), (w2t, w2t_d), (w2b, w2b_d)]:
                nc.sync.dma_start(t[:], d[:])
            for t, d in [(w4t, w4t_d), (w4b, w4b_d)]:
                nc.scalar.dma_start(t[:], d[:])
            for t, d in [(encmm, encmm_d), (w1t, w1t_d), (w3t, w3t_d),
                         (w1b, w1b_d), (w3b, w3b_d), (zmm, zmm_d),
                         (encrep, encrep_d)]:
                nc.scalar.dma_start(t[:], d[:])

            ones = wpool.tile([128, 128], f16)
            nc.vector.memset(ones[:], 1.0)

            # ---- linears: U/V first (they feed the per-tc bcast DMAs) ----
            psum_ctx = tc.tile_pool(name="psum", bufs=2, space="PSUM")
            psum = psum_ctx.__enter__()
            A_nat = apool.tile([128, NBLK, DH], f16)
            C_nat = apool.tile([128, NBLK, DH], f16)
            A_rep = apool.tile([128, NBLK, CH], f16)
            C_rep = apool.tile([128, NBLK, CH], f16)
            U_td = apool.tile([T, DH], f16)
            V_td = apool.tile([T, DH], f16)

            pu = psum.tile([T, DH], f32)
            for k in range(4):
                nc.tensor.matmul(pu[:], outmm[:, k, :], w2t[:, k, :],
                                 start=(k == 0), stop=False)
            nc.tensor.matmul(pu[:], ones[0:1, 0:T], w2b[:], start=False, stop=True)
            nc.scalar.activation(out=U_td[:], in_=pu[:], func=Tanh)

            pv = psum.tile([T, DH], f32)
            for k in range(4):
                nc.tensor.matmul(pv[:], outmm[:, k, :], w4t[:, k, :],
                                 start=(k == 0), stop=False)
            nc.tensor.matmul(pv[:], ones[0:1, 0:T], w4b[:], start=False, stop=True)
            nc.scalar.activation(out=V_td[:], in_=pv[:], func=Tanh)

            dram_u = dpool.tile([T, DH], f16)
            dram_v = dpool.tile([T, DH], f16)
            nc.sync.dma_start(dram_u[:], U_td[:])
            nc.sync.dma_start(dram_v[:], V_td[:])

            for sblk in range(NBLK):
                scols = bass.ts(sblk, 128)
                pa = psum.tile([128, DH], f32)
                for k in range(4):
                    nc.tensor.matmul(pa[:], encmm[:, k, scols], w1t[:, k, :],
                                     start=(k == 0), stop=False)
                nc.tensor.matmul(pa[:], ones[0:1, 0:128], w1b[:],
                                 start=False, stop=True)
                nc.scalar.activation(out=A_nat[:, sblk, :], in_=pa[:], func=Tanh)

                pc = psum.tile([128, DH], f32)
                nc.tensor.matmul(pc[:], zmm[:, scols], w3t[:], start=True,
                                 stop=False)
                nc.tensor.matmul(pc[:], ones[0:1, 0:128], w3b[:],
                                 start=False, stop=True)
                nc.scalar.activation(out=C_nat[:, sblk, :], in_=pc[:], func=Tanh)
                # replicate this sblk's A/C right away (step-0 free-dim read)
                for rep_dst, nat_src in [(A_rep, A_nat), (C_rep, C_nat)]:
                    src_ap = bass.AP(
                        tensor=nat_src.tensor,
                        offset=nat_src.offset + sblk * DH,
                        ap=[nat_src.ap[0], [0, TCH], [1, DH]],
                    )
                    nc.sync.dma_start(rep_dst[:, sblk, :].rearrange(
                        "p (tch dh) -> p tch dh", tch=TCH), src_ap)
            psum_ctx.__exit__(None, None, None)
            psum_r_ctx = tc.tile_pool(name="psum_r", bufs=2, space="PSUM")
            psum_r = psum_r_ctx.__enter__()

            # ---- gate loop ----
            numsc = spool.tile([T, DH], f32, tag="numsc")
            densc = spool.tile([T, DH], f32, tag="densc")
            def flush(pend):
                # P-mult + PE reduces for a completed (tc, sblk); stage is
                # software-pipelined one iteration behind the exp producer.
                Ec_p, tcp, sbp, pnd_p = pend
                Pc = gpool.tile([128, CH], f16, tag="Pc")
                nc.vector.tensor_tensor(out=Pc[:], in0=Ec_p[:],
                                        in1=encrep[:, sbp, :], op=MUL)
                for sl in range(CH // 512):
                    cs = bass.ts(sl, 512)
                    nc.tensor.matmul(pnd_p[0:1, cs], ones[:, 0:1], Ec_p[:, cs],
                                     start=(sbp == 0), stop=(sbp == 1))
                    nc.tensor.matmul(pnd_p[32:33, cs], ones[:, 0:1], Pc[:, cs],
                                     start=(sbp == 0), stop=(sbp == 1))
                if sbp == NBLK - 1:
                    ndrow = spool.tile([33, CH], f32, tag="ndrow")
                    nc.scalar.copy(ndrow[:], pnd_p[0:33, :])
                    trows = slice(tcp * TCH, (tcp + 1) * TCH)
                    nc.sync.dma_start(
                        numsc[trows, :],
                        ndrow[32:33, :].rearrange("p (t d) -> p t d", t=TCH))
                    nc.sync.dma_start(
                        densc[trows, :],
                        ndrow[0:1, :].rearrange("p (t d) -> p t d", t=TCH))

            pend = None
            for tc_i in range(NTC):
                u_bc = bcpool.tile([128, CH], f16, tag="u_bc")
                v_bc = bcpool.tile([128, CH], f16, tag="v_bc")
                usrc = bass.AP(tensor=dram_u.tensor,
                               offset=dram_u.offset + tc_i * CH,
                               ap=[[0, 128], [1, CH]])
                vsrc = bass.AP(tensor=dram_v.tensor,
                               offset=dram_v.offset + tc_i * CH,
                               ap=[[0, 128], [1, CH]])
                nc.sync.dma_start(u_bc[:], usrc)
                nc.sync.dma_start(v_bc[:], vsrc)

                # den -> PSUM partition 0, num -> partition 32 of ONE
                # 4-bank tile: single ACT copy reads both; 2 bufs pipeline.
                # Both sblocks interleaved so no DVE op reads the output of
                # the immediately-preceding DVE op (drain serialization).
                pnd = psum_r.tile([64, CH], f32, tag="pnd")
                m1a = gpool.tile([128, CH], f16, tag="m1")
                m2a = gpool.tile([128, CH], f16, tag="m2")
                Eca = gpool.tile([128, CH], f16, tag="Ec")
                m1b = gpool.tile([128, CH], f16, tag="m1")
                m2b = gpool.tile([128, CH], f16, tag="m2")
                Ecb = gpool.tile([128, CH], f16, tag="Ec")
                nc.vector.tensor_tensor(out=m1a[:], in0=A_rep[:, 0, :],
                                        in1=u_bc[:], op=MUL)
                nc.vector.tensor_tensor(out=m2a[:], in0=C_rep[:, 0, :],
                                        in1=v_bc[:], op=MUL)
                nc.vector.tensor_tensor(out=m1b[:], in0=A_rep[:, 1, :],
                                        in1=u_bc[:], op=MUL)
                nc.vector.tensor_tensor(out=m2b[:], in0=C_rep[:, 1, :],
                                        in1=v_bc[:], op=MUL)
                nc.vector.tensor_tensor(out=m1a[:], in0=m1a[:], in1=m2a[:],
                                        op=ADD)
                nc.vector.tensor_tensor(out=m1b[:], in0=m1b[:], in1=m2b[:],
                                        op=ADD)
                nc.scalar.activation(out=Eca[:], in_=m1a[:],
                                     func=mybir.ActivationFunctionType.Exp)
                nc.scalar.activation(out=Ecb[:], in_=m1b[:],
                                     func=mybir.ActivationFunctionType.Exp)
                if pend is not None:
                    flush(pend[0])
                    flush(pend[1])
                pend = ((Eca, tc_i, 0, pnd), (Ecb, tc_i, 1, pnd))
            flush(pend[0])
            flush(pend[1])

            TH = T // 2
            rden = spool.tile([T, DH], f32, tag="rden")
            attn_s = spool.tile([T, DH], f32, tag="attn_s")
            for half in range(2):
                hr = slice(half * TH, (half + 1) * TH)
                nc.vector.reciprocal(out=rden[hr, :], in_=densc[hr, :])
                nc.vector.tensor_tensor(out=attn_s[hr, :], in0=numsc[hr, :],
                                        in1=rden[hr, :], op=MUL)
                nc.sync.dma_start(attn_d[hr, :], attn_s[hr, :])
            psum_r_ctx.__exit__(None, None, None)

    nc.finalize()
    return nc


def _get_program():
    if os.environ.get("K_LAYOUT", "s") == "s":
        if "s" not in _prog_cache:
            _prog_cache["s"] = _build_program_s()
        return _prog_cache["s"]
    key = (os.environ.get("K_DEN", "act"), os.environ.get("K_L", "tt"))
    if key not in _prog_cache:
        _prog_cache[key] = _build_program_d(den_mode=key[0], lmode=key[1])
    return _prog_cache[key]


def _host_prep(output, encoder_hidden, input_z, W1, b1, W2, b2, W3, b3, W4, b4):
    """Layout-only transforms -> per-core input maps."""
    f16 = np.float16
    H = np.ascontiguousarray(encoder_hidden).reshape(S * B, E)
    W1T = W1.T.astype(f16)   # [E, D]
    W2T = W2.T.astype(f16)
    W4T = W4.T.astype(f16)
    W3T = W3.T.astype(f16)   # [F, D]
    slayout = os.environ.get("K_LAYOUT", "s") == "s"
    in_maps = []
    for c in range(N_CORES):
        b, h = divmod(c, 2)
        dcols = slice(h * DH, (h + 1) * DH)
        slab = H[b * S:(b + 1) * S]                       # [S, E]
        encmm = np.ascontiguousarray(slab.T).astype(f16)  # [E, S]

        def kmajor(x, k):          # [k*128, X] -> [128, k, X]
            return np.ascontiguousarray(x.reshape(k, 128, -1).transpose(1, 0, 2))

        if slayout:
            encs = slab[:, dcols].astype(f16)              # [S, DH]
            encrep = np.ascontiguousarray(
                np.tile(encs.reshape(2, 128, 1, DH), (1, 1, TCH, 1))
                .transpose(1, 0, 2, 3)                     # [128, NBLK, TCH, DH]
            ).reshape(128, NBLK, TCH * DH)
            in_maps.append({
                "w1t": kmajor(W1T[:, dcols], 4),
                "w2t": kmajor(W2T[:, dcols], 4),
                "w4t": kmajor(W4T[:, dcols], 4),
                "w3t": np.ascontiguousarray(W3T[:, dcols]),
                "w1b": b1[dcols].reshape(1, DH).astype(f16),
                "w2b": b2[dcols].reshape(1, DH).astype(f16),
                "w3b": b3[dcols].reshape(1, DH).astype(f16),
                "w4b": b4[dcols].reshape(1, DH).astype(f16),
                "encmm": kmajor(encmm, 4),
                "outmm": kmajor(np.ascontiguousarray(output[b].T).astype(f16), 4),
                "zmm": np.ascontiguousarray(input_z[b].T).astype(f16),
                "encrep": encrep,
            })
            continue
        in_maps.append({
            "w1t": kmajor(W1T[:, dcols], 4),
            "w2t": kmajor(W2T[:, dcols], 4),
            "w4t": kmajor(W4T[:, dcols], 4),
            "w3t": np.ascontiguousarray(W3T[:, dcols]),
            "encmm": kmajor(encmm, 4),
            "encg": kmajor(encmm[h * DH:(h + 1) * DH], NBLK),
            "outmm": kmajor(np.ascontiguousarray(output[b].T).astype(f16), 4),
            "zmm": np.ascontiguousarray(input_z[b].T).astype(f16),
            "b1": np.ascontiguousarray(b1[dcols].reshape(NBLK, 128).T).astype(np.float32),
            "b2": np.ascontiguousarray(b2[dcols].reshape(NBLK, 128).T).astype(np.float32),
            "b3": np.ascontiguousarray(b3[dcols].reshape(NBLK, 128).T).astype(np.float32),
            "b4": np.ascontiguousarray(b4[dcols].reshape(NBLK, 128).T).astype(np.float32),
        })
    return in_maps


def _install_ntff_hook():
    """The agent image's antenv lacks axon_hooks; synthesize it so
    run_bass_kernel_spmd(trace=True) can collect NTFF profiles."""
    import types

    if "antenv.axon_hooks" in sys.modules:
        return
    import antenv

    mod = types.ModuleType("antenv.axon_hooks")
    holder = {"h": None}
    mod.set_axon_ntff_profile_hook = lambda h: holder.__setitem__("h", h)
    mod.get_axon_ntff_profile_hook = lambda: holder["h"]
    sys.modules["antenv.axon_hooks"] = mod
    antenv.axon_hooks = mod
    try:
        from trn_agent_boot.trn_boot import _ntff_profile_via_ctypes

        h = _ntff_profile_via_ctypes("/opt/axon/libaxon_pjrt.so")
        if h is not None:
            mod.set_axon_ntff_profile_hook(h)
    except Exception as e:
        print(f"ntff hook install failed: {e}", file=sys.stderr)


def _run(inputs, trace=False):
    from concourse.bass_utils import run_bass_kernel_spmd

    if trace:
        try:
            _install_ntff_hook()
        except Exception as e:
            print(f"ntff hook unavailable: {e}", file=sys.stderr)

    nc = _get_program()
    in_maps = _host_prep(**inputs)
    res = run_bass_kernel_spmd(
        nc, in_maps, core_ids=list(range(N_CORES)), trace=trace,
    )
    output = np.asarray(inputs["output"], dtype=np.float32)
    attn = np.empty((B, T, D), np.float32)
    slayout = os.environ.get("K_LAYOUT", "s") == "s"
    for c in range(N_CORES):
        b, h = divmod(c, 2)
        if slayout:
            attn[b, :, h * DH:(h + 1) * DH] = res.results[c]["attn_s"]
        else:
            at = res.results[c]["attn_t"]                 # [128, NBLK, T]
            at = at.transpose(1, 0, 2).reshape(DH, T)     # [d, t]
            attn[b, :, h * DH:(h + 1) * DH] = at.T
    concat = np.concatenate([output, attn], axis=-1)
    return (concat, attn), res


def kernel(**inputs):
    inputs = {k: np.asarray(v) for k, v in inputs.items()}
    (concat, attn), _ = _run(inputs, trace=False)
    return concat, attn


# revision 34
# speedup vs baseline: 1.1138x; 1.0397x over previous
"""Trainium2 Bass kernel for nn_DualAttention (8 NeuronCores).

Math: the reference's alpha/beta double-softmax collapses to a single
softmax:  gamma = softmax_s(u_d*A[s,d] + v_d*C[s,d]) with
  A = tanh(lin1(enc)) (raw-view reshaped), C = tanh(lin3(z)),
  u = tanh(lin2(out)), v = tanh(lin4(out)),
  attn[b,t,d] = sum_s gamma * enc_r[b,s,d].
The raw [S,B,E] -> [B,S,E] view means batch b's slab is
encoder_hidden.reshape(S*B, E)[b*S:(b+1)*S].

Sharding: core c -> (b = c//2, d-half h = c%2).  Each core computes
attn for its (b, 256 d-values) over all T=64 timesteps.

Two device layouts (K_LAYOUT env, default "s"):
 - "s": s on partitions. Gate ops are chunked fp16 tensor_tensor at 2x
   rate; softmax reductions over s run on the Tensor engine as
   ones-matmuls over partitions; u/v broadcast via DRAM step-0 DMA.
 - "d": d on partitions. Per-t tensor_scalar/scalar_tensor_tensor +
   affine_mul_reduce; den via ACT exp accum.
Host prep is layout/dtype only (transposes + fp16 casts + slicing).
"""

import os
import sys

sys.path.insert(0, "/opt/trn_rl_repo")

import numpy as np

B, T, S, D, E, F = 4, 64, 256, 512, 512, 128
DH = D // 2          # d per core
NBLK = 2             # 128-row blocks per core (d-blocks or s-blocks)
TCH = 8              # t chunk
N_CORES = 8

_prog_cache = {}


def _build_program_d(den_mode="act", lmode="tt"):
    import concourse.bass as bass
    import concourse.tile as tile
    from concourse import bacc, mybir

    f16 = mybir.dt.float16
    f32 = mybir.dt.float32
    MUL = mybir.AluOpType.mult
    ADD = mybir.AluOpType.add

    nc = bacc.Bacc(None, target_bir_lowering=False)

    w1t_d = nc.dram_tensor("w1t", [128, 4, DH], f16, kind="ExternalInput")
    w2t_d = nc.dram_tensor("w2t", [128, 4, DH], f16, kind="ExternalInput")
    w4t_d = nc.dram_tensor("w4t", [128, 4, DH], f16, kind="ExternalInput")
    w3t_d = nc.dram_tensor("w3t", [128, DH], f16, kind="ExternalInput")
    encmm_d = nc.dram_tensor("encmm", [128, 4, S], f16, kind="ExternalInput")
    encg_d = nc.dram_tensor("encg", [128, NBLK, S], f16, kind="ExternalInput")
    outmm_d = nc.dram_tensor("outmm", [128, 4, T], f16, kind="ExternalInput")
    zmm_d = nc.dram_tensor("zmm", [128, S], f16, kind="ExternalInput")
    b1_d = nc.dram_tensor("b1", [128, NBLK], f32, kind="ExternalInput")
    b2_d = nc.dram_tensor("b2", [128, NBLK], f32, kind="ExternalInput")
    b3_d = nc.dram_tensor("b3", [128, NBLK], f32, kind="ExternalInput")
    b4_d = nc.dram_tensor("b4", [128, NBLK], f32, kind="ExternalInput")
    attn_d = nc.dram_tensor("attn_t", [128, NBLK, T], f32, kind="ExternalOutput")

    with tile.TileContext(nc) as tc:
        with (
            tc.tile_pool(name="weights", bufs=1) as wpool,
            tc.tile_pool(name="acts", bufs=1) as apool,
            tc.tile_pool(name="gate", bufs=3) as gpool,
            tc.tile_pool(name="scr", bufs=4) as spool,
            tc.tile_pool(name="psum", bufs=2, space="PSUM") as psum,
            tc.tile_pool(name="psum_uv", bufs=2, space="PSUM") as psum_uv,
        ):
            w1t = wpool.tile([128, 4, DH], f16)
            w2t = wpool.tile([128, 4, DH], f16)
            w4t = wpool.tile([128, 4, DH], f16)
            w3t = wpool.tile([128, DH], f16)
            encmm = wpool.tile([128, 4, S], f16)
            encg = wpool.tile([128, NBLK, S], f16)
            outmm = wpool.tile([128, 4, T], f16)
            zmm = wpool.tile([128, S], f16)
            b1s = wpool.tile([128, NBLK], f32)
            b2s = wpool.tile([128, NBLK], f32)
            b3s = wpool.tile([128, NBLK], f32)
            b4s = wpool.tile([128, NBLK], f32)
            for t, d in [(w1t, w1t_d), (w2t, w2t_d), (w4t, w4t_d), (w3t, w3t_d),
                         (encmm, encmm_d), (encg, encg_d), (outmm, outmm_d),
                         (zmm, zmm_d), (b1s, b1_d), (b2s, b2_d), (b3s, b3_d),
                         (b4s, b4_d)]:
                nc.sync.dma_start(t[:], d[:])

            A = apool.tile([128, NBLK, S], f16)
            C = apool.tile([128, NBLK, S], f16)
            U = apool.tile([128, NBLK, T], f32)
            V = apool.tile([128, NBLK, T], f32)
            Tanh = mybir.ActivationFunctionType.Tanh

            for blk in range(NBLK):
                dcols = bass.ts(blk, 128)
                pa = psum.tile([128, S], f32)
                for k in range(4):
                    nc.tensor.matmul(pa[:], w1t[:, k, dcols], encmm[:, k, :],
                                     start=(k == 0), stop=(k == 3))
                nc.scalar.activation(out=A[:, blk, :], in_=pa[:], func=Tanh,
                                     bias=b1s[:, blk:blk + 1], scale=1.0)

                pc = psum.tile([128, S], f32)
                nc.tensor.matmul(pc[:], w3t[:, dcols], zmm[:], start=True, stop=True)
                nc.scalar.activation(out=C[:, blk, :], in_=pc[:], func=Tanh,
                                     bias=b3s[:, blk:blk + 1], scale=1.0)

                pu = psum_uv.tile([128, T], f32)
                for k in range(4):
                    nc.tensor.matmul(pu[:], w2t[:, k, dcols], outmm[:, k, :],
                                     start=(k == 0), stop=(k == 3))
                nc.scalar.activation(out=U[:, blk, :], in_=pu[:], func=Tanh,
                                     bias=b2s[:, blk:blk + 1], scale=1.0)

                pv = psum_uv.tile([128, T], f32)
                for k in range(4):
                    nc.tensor.matmul(pv[:], w4t[:, k, dcols], outmm[:, k, :],
                                     start=(k == 0), stop=(k == 3))
                nc.scalar.activation(out=V[:, blk, :], in_=pv[:], func=Tanh,
                                     bias=b4s[:, blk:blk + 1], scale=1.0)

            num = apool.tile([128, NBLK, T], f32, tag="num")
            den = apool.tile([128, NBLK, T], f32, tag="den")
            for blk in range(NBLK):
                for tc_i in range(T // TCH):
                    m2c = gpool.tile([128, TCH, S], f16, tag="m2c")
                    Lc = gpool.tile([128, TCH, S], f16, tag="Lc")
                    Ec = gpool.tile([128, TCH, S], f16, tag="Ec")
                    m1c = None
                    if lmode == "tt":
                        m1c = gpool.tile([128, TCH, S], f16, tag="m1c")
                    for j in range(TCH):
                        t = tc_i * TCH + j
                        nc.vector.tensor_scalar(
                            out=m2c[:, j, :], in0=C[:, blk, :],
                            scalar1=V[:, blk, t:t + 1], scalar2=None, op0=MUL)
                        if lmode == "tt":
                            nc.vector.tensor_scalar(
                                out=m1c[:, j, :], in0=A[:, blk, :],
                                scalar1=U[:, blk, t:t + 1], scalar2=None, op0=MUL)
                        else:
                            nc.vector.scalar_tensor_tensor(
                                out=Lc[:, j, :], in0=A[:, blk, :],
                                scalar=U[:, blk, t:t + 1], in1=m2c[:, j, :],
                                op0=MUL, op1=ADD)
                    if lmode == "tt":
                        nc.vector.tensor_tensor(out=Lc[:], in0=m1c[:], in1=m2c[:],
                                                op=ADD)
                    if den_mode == "act":
                        for j in range(TCH):
                            t = tc_i * TCH + j
                            nc.scalar.activation(
                                out=Ec[:, j, :], in_=Lc[:, j, :],
                                func=mybir.ActivationFunctionType.Exp,
                                accum_out=den[:, blk, t:t + 1])
                    else:
                        nc.scalar.activation(out=Ec[:], in_=Lc[:],
                                             func=mybir.ActivationFunctionType.Exp)
                        nc.vector.tensor_reduce(
                            out=den[:, blk, tc_i * TCH:(tc_i + 1) * TCH],
                            in_=Ec[:], axis=mybir.AxisListType.X, op=ADD)
                    for j in range(TCH):
                        t = tc_i * TCH + j
                        pscr = spool.tile([128, S], f16, tag="pscr")
                        nc.vector.affine_mul_reduce(
                            out=pscr[:], accum_out=num[:, blk, t:t + 1],
                            in0=Ec[:, j, :], in1=encg[:, blk, :], scale=1.0,
                            bias=0.0)

                rden = spool.tile([128, T], f32, tag="rden")
                attn_t = spool.tile([128, T], f32, tag="attn")
                nc.vector.reciprocal(out=rden[:], in_=den[:, blk, :])
                nc.vector.tensor_tensor(out=attn_t[:], in0=num[:, blk, :],
                                        in1=rden[:], op=MUL)
                nc.sync.dma_start(attn_d[:, blk, :], attn_t[:])

    nc.finalize()
    return nc


def _build_program_s():
    """s-on-partitions layout."""
    import concourse.bass as bass
    import concourse.tile as tile
    from concourse import bacc, mybir

    f16 = mybir.dt.float16
    f32 = mybir.dt.float32
    MUL = mybir.AluOpType.mult
    ADD = mybir.AluOpType.add
    Tanh = mybir.ActivationFunctionType.Tanh
    NTC = T // TCH          # 8 t-chunks
    CH = TCH * DH           # 2048 free per chunk

    nc = bacc.Bacc(None, target_bir_lowering=False)

    w1t_d = nc.dram_tensor("w1t", [128, 4, DH], f16, kind="ExternalInput")
    w2t_d = nc.dram_tensor("w2t", [128, 4, DH], f16, kind="ExternalInput")
    w4t_d = nc.dram_tensor("w4t", [128, 4, DH], f16, kind="ExternalInput")
    w3t_d = nc.dram_tensor("w3t", [128, DH], f16, kind="ExternalInput")
    w1b_d = nc.dram_tensor("w1b", [1, DH], f16, kind="ExternalInput")
    w2b_d = nc.dram_tensor("w2b", [1, DH], f16, kind="ExternalInput")
    w3b_d = nc.dram_tensor("w3b", [1, DH], f16, kind="ExternalInput")
    w4b_d = nc.dram_tensor("w4b", [1, DH], f16, kind="ExternalInput")
    encmm_d = nc.dram_tensor("encmm", [128, 4, S], f16, kind="ExternalInput")
    outmm_d = nc.dram_tensor("outmm", [128, 4, T], f16, kind="ExternalInput")
    zmm_d = nc.dram_tensor("zmm", [128, S], f16, kind="ExternalInput")
    encrep_d = nc.dram_tensor("encrep", [128, NBLK, CH], f16, kind="ExternalInput")
    attn_d = nc.dram_tensor("attn_s", [T, DH], f32, kind="ExternalOutput")

    with tile.TileContext(nc) as tc:
        with (
            tc.tile_pool(name="weights", bufs=1) as wpool,
            tc.tile_pool(name="acts", bufs=1) as apool,
            tc.tile_pool(name="bc", bufs=6) as bcpool,
            tc.tile_pool(name="gate", bufs=4) as gpool,
            tc.tile_pool(name="scr", bufs=2) as spool,
            tc.tile_pool(name="dram", bufs=2, space="DRAM") as dpool,
        ):
            w1t = wpool.tile([128, 4, DH], f16)
            w2t = wpool.tile([128, 4, DH], f16)
            w4t = wpool.tile([128, 4, DH], f16)
            w3t = wpool.tile([128, DH], f16)
            w1b = wpool.tile([1, DH], f16)
            w2b = wpool.tile([1, DH], f16)
            w3b = wpool.tile([1, DH], f16)
            w4b = wpool.tile([1, DH], f16)
            encmm = wpool.tile([128, 4, S], f16)
            outmm = wpool.tile([128, 4, T], f16)
            zmm = wpool.tile([128, S], f16)
            encrep = wpool.tile([128, NBLK, CH], f16)
            for t, d in [(outmm, outmm_d), (w2t, w2t_d), (w2b, w2b_d)]:
                nc.sync.dma_start(t[:], d[:])
            for t, d in [(w4t, w4t_d), (w4b, w4b_d)]:
                nc.scalar.dma_start(t[:], d[:])
            for t, d in [(encmm, encmm_d), (w1t, w1t_d), (w3t, w3t_d),
                         (w1b, w1b_d), (w3b, w3b_d), (zmm, zmm_d),
                         (encrep, encrep_d)]:
                nc.scalar.dma_start(t[:], d[:])

            ones = wpool.tile([128, 128], f16)
            nc.vector.memset(ones[:], 1.0)

            # ---- linears: U/V first (they feed the per-tc bcast DMAs) ----
            psum_ctx = tc.tile_pool(name="psum", bufs=2, space="PSUM")
            psum = psum_ctx.__enter__()
            A_nat = apool.tile([128, NBLK, DH], f16)
            C_nat = apool.tile([128, NBLK, DH], f16)
            A_rep = apool.tile([128, NBLK, CH], f16)
            C_rep = apool.tile([128, NBLK, CH], f16)
            U_td = apool.tile([T, DH], f16)
            V_td = apool.tile([T, DH], f16)

            pu = psum.tile([T, DH], f32)
            for k in range(4):
                nc.tensor.matmul(pu[:], outmm[:, k, :], w2t[:, k, :],
                                 start=(k == 0), stop=False)
            nc.tensor.matmul(pu[:], ones[0:1, 0:T], w2b[:], start=False, stop=True)
            nc.scalar.activation(out=U_td[:], in_=pu[:], func=Tanh)

            pv = psum.tile([T, DH], f32)
            for k in range(4):
                nc.tensor.matmul(pv[:], outmm[:, k, :], w4t[:, k, :],
                                 start=(k == 0), stop=False)
            nc.tensor.matmul(pv[:], ones[0:1, 0:T], w4b[:], start=False, stop=True)
            nc.scalar.activation(out=V_td[:], in_=pv[:], func=Tanh)

            dram_u = dpool.tile([T, DH], f16)
            dram_v = dpool.tile([T, DH], f16)
            nc.sync.dma_start(dram_u[:], U_td[:])
            nc.sync.dma_start(dram_v[:], V_td[:])

            for sblk in range(NBLK):
                scols = bass.ts(sblk, 128)
                pa = psum.tile([128, DH], f32)
                for k in range(4):
                    nc.tensor.matmul(pa[:], encmm[:, k, scols], w1t[:, k, :],
                                     start=(k == 0), stop=False)
                nc.tensor.matmul(pa[:], ones[0:1, 0:128], w1b[:],
                                 start=False, stop=True)
                nc.scalar.activation(out=A_nat[:, sblk, :], in_=pa[:], func=Tanh)

                pc = psum.tile([128, DH], f32)
                nc.tensor.matmul(pc[:], zmm[:, scols], w3t[:], start=True,
                                 stop=False)
                nc.tensor.matmul(pc[:], ones[0:1, 0:128], w3b[:],
                                 start=False, stop=True)
                nc.scalar.activation(out=C_nat[:, sblk, :], in_=pc[:], func=Tanh)
                # replicate this sblk's A/C right away (step-0 free-dim read)
                for rep_dst, nat_src in [(A_rep, A_nat), (C_rep, C_nat)]:
                    src_ap = bass.AP(
                        tensor=nat_src.tensor,
                        offset=nat_src.offset + sblk * DH,
                        ap=[nat_src.ap[0], [0, TCH], [1, DH]],
                    )
                    nc.sync.dma_start(rep_dst[:, sblk, :].rearrange(
                        "p (tch dh) -> p tch dh", tch=TCH), src_ap)
                if sblk == 0:
                    # issue tc=0 broadcasts now so they aren't queued behind
                    # the sblk-1 replication transfers (in-order DMA queue)
                    u_bc0 = bcpool.tile([128, CH], f16, tag="u_bc")
                    v_bc0 = bcpool.tile([128, CH], f16, tag="v_bc")
                    nc.sync.dma_start(u_bc0[:], bass.AP(
                        tensor=dram_u.tensor, offset=dram_u.offset,
                        ap=[[0, 128], [1, CH]]))
                    nc.sync.dma_start(v_bc0[:], bass.AP(
                        tensor=dram_v.tensor, offset=dram_v.offset,
                        ap=[[0, 128], [1, CH]]))
            psum_ctx.__exit__(None, None, None)
            psum_r_ctx = tc.tile_pool(name="psum_r", bufs=2, space="PSUM")
            psum_r = psum_r_ctx.__enter__()

            # ---- gate loop ----
            numsc = spool.tile([T, DH], f32, tag="numsc")
            densc = spool.tile([T, DH], f32, tag="densc")
            def flush(pend):
                # P-mult + PE reduces for a completed (tc, sblk); stage is
                # software-pipelined one iteration behind the exp producer.
                Ec_p, tcp, sbp, pnd_p = pend
                Pc = gpool.tile([128, CH], f16, tag="Pc")
                nc.vector.tensor_tensor(out=Pc[:], in0=Ec_p[:],
                                        in1=encrep[:, sbp, :], op=MUL)
                for sl in range(CH // 512):
                    cs = bass.ts(sl, 512)
                    nc.tensor.matmul(pnd_p[0:1, cs], ones[:, 0:1], Ec_p[:, cs],
                                     start=(sbp == 0), stop=(sbp == 1))
                    nc.tensor.matmul(pnd_p[32:33, cs], ones[:, 0:1], Pc[:, cs],
                                     start=(sbp == 0), stop=(sbp == 1))
                if sbp == NBLK - 1:
                    ndrow = spool.tile([33, CH], f32, tag="ndrow")
                    nc.scalar.copy(ndrow[:], pnd_p[0:33, :])
                    trows = slice(tcp * TCH, (tcp + 1) * TCH)
                    nc.sync.dma_start(
                        numsc[trows, :],
                        ndrow[32:33, :].rearrange("p (t d) -> p t d", t=TCH))
                    nc.sync.dma_start(
                        densc[trows, :],
                        ndrow[0:1, :].rearrange("p (t d) -> p t d", t=TCH))

            pend = None
            for tc_i in range(NTC):
                if tc_i == 0:
                    u_bc, v_bc = u_bc0, v_bc0
                else:
                    u_bc = bcpool.tile([128, CH], f16, tag="u_bc")
                    v_bc = bcpool.tile([128, CH], f16, tag="v_bc")
                    usrc = bass.AP(tensor=dram_u.tensor,
                                   offset=dram_u.offset + tc_i * CH,
                                   ap=[[0, 128], [1, CH]])
                    vsrc = bass.AP(tensor=dram_v.tensor,
                                   offset=dram_v.offset + tc_i * CH,
                                   ap=[[0, 128], [1, CH]])
                    nc.sync.dma_start(u_bc[:], usrc)
                    nc.sync.dma_start(v_bc[:], vsrc)

                # den -> PSUM partition 0, num -> partition 32 of ONE
                # 4-bank tile: single ACT copy reads both; 2 bufs pipeline.
                # Both sblocks interleaved so no DVE op reads the output of
                # the immediately-preceding DVE op (drain serialization).
                pnd = psum_r.tile([64, CH], f32, tag="pnd")
                m1a = gpool.tile([128, CH], f16, tag="m1")
                m2a = gpool.tile([128, CH], f16, tag="m2")
                Eca = gpool.tile([128, CH], f16, tag="Ec")
                m1b = gpool.tile([128, CH], f16, tag="m1")
                m2b = gpool.tile([128, CH], f16, tag="m2")
                Ecb = gpool.tile([128, CH], f16, tag="Ec")
                nc.vector.tensor_tensor(out=m1a[:], in0=A_rep[:, 0, :],
                                        in1=u_bc[:], op=MUL)
                nc.vector.tensor_tensor(out=m2a[:], in0=C_rep[:, 0, :],
                                        in1=v_bc[:], op=MUL)
                nc.vector.tensor_tensor(out=m1b[:], in0=A_rep[:, 1, :],
                                        in1=u_bc[:], op=MUL)
                nc.vector.tensor_tensor(out=m2b[:], in0=C_rep[:, 1, :],
                                        in1=v_bc[:], op=MUL)
                nc.vector.tensor_tensor(out=m1a[:], in0=m1a[:], in1=m2a[:],
                                        op=ADD)
                nc.vector.tensor_tensor(out=m1b[:], in0=m1b[:], in1=m2b[:],
                                        op=ADD)
                nc.scalar.activation(out=Eca[:], in_=m1a[:],
                                     func=mybir.ActivationFunctionType.Exp)
                nc.scalar.activation(out=Ecb[:], in_=m1b[:],
                                     func=mybir.ActivationFunctionType.Exp)
                if pend is not None:
                    flush(pend[0])
                    flush(pend[1])
                pend = ((Eca, tc_i, 0, pnd), (Ecb, tc_i, 1, pnd))
            flush(pend[0])
            flush(pend[1])

            TH = T // 2
            rden = spool.tile([T, DH], f32, tag="rden")
            attn_s = spool.tile([T, DH], f32, tag="attn_s")
            for half in range(2):
                hr = slice(half * TH, (half + 1) * TH)
                nc.vector.reciprocal(out=rden[hr, :], in_=densc[hr, :])
                nc.vector.tensor_tensor(out=attn_s[hr, :], in0=numsc[hr, :],
                                        in1=rden[hr, :], op=MUL)
                nc.sync.dma_start(attn_d[hr, :], attn_s[hr, :])
            psum_r_ctx.__exit__(None, None, None)

    nc.finalize()
    return nc


def _get_program():
    if os.environ.get("K_LAYOUT", "s") == "s":
        if "s" not in _prog_cache:
            _prog_cache["s"] = _build_program_s()
        return _prog_cache["s"]
    key = (os.environ.get("K_DEN", "act"), os.environ.get("K_L", "tt"))
    if key not in _prog_cache:
        _prog_cache[key] = _build_program_d(den_mode=key[0], lmode=key[1])
    return _prog_cache[key]


def _host_prep(output, encoder_hidden, input_z, W1, b1, W2, b2, W3, b3, W4, b4):
    """Layout-only transforms -> per-core input maps."""
    f16 = np.float16
    H = np.ascontiguousarray(encoder_hidden).reshape(S * B, E)
    W1T = W1.T.astype(f16)   # [E, D]
    W2T = W2.T.astype(f16)
    W4T = W4.T.astype(f16)
    W3T = W3.T.astype(f16)   # [F, D]
    slayout = os.environ.get("K_LAYOUT", "s") == "s"
    in_maps = []
    for c in range(N_CORES):
        b, h = divmod(c, 2)
        dcols = slice(h * DH, (h + 1) * DH)
        slab = H[b * S:(b + 1) * S]                       # [S, E]
        encmm = np.ascontiguousarray(slab.T).astype(f16)  # [E, S]

        def kmajor(x, k):          # [k*128, X] -> [128, k, X]
            return np.ascontiguousarray(x.reshape(k, 128, -1).transpose(1, 0, 2))

        if slayout:
            encs = slab[:, dcols].astype(f16)              # [S, DH]
            encrep = np.ascontiguousarray(
                np.tile(encs.reshape(2, 128, 1, DH), (1, 1, TCH, 1))
                .transpose(1, 0, 2, 3)                     # [128, NBLK, TCH, DH]
            ).reshape(128, NBLK, TCH * DH)
            in_maps.append({
                "w1t": kmajor(W1T[:, dcols], 4),
                "w2t": kmajor(W2T[:, dcols], 4),
                "w4t": kmajor(W4T[:, dcols], 4),
                "w3t": np.ascontiguousarray(W3T[:, dcols]),
                "w1b": b1[dcols].reshape(1, DH).astype(f16),
                "w2b": b2[dcols].reshape(1, DH).astype(f16),
                "w3b": b3[dcols].reshape(1, DH).astype(f16),
                "w4b": b4[dcols].reshape(1, DH).astype(f16),
                "encmm": kmajor(encmm, 4),
                "outmm": kmajor(np.ascontiguousarray(output[b].T).astype(f16), 4),
                "zmm": np.ascontiguousarray(input_z[b].T).astype(f16),
                "encrep": encrep,
            })
            continue
        in_maps.append({
            "w1t": kmajor(W1T[:, dcols], 4),
            "w2t": kmajor(W2T[:, dcols], 4),
            "w4t": kmajor(W4T[:, dcols], 4),
            "w3t": np.ascontiguousarray(W3T[:, dcols]),
            "encmm": kmajor(encmm, 4),
            "encg": kmajor(encmm[h * DH:(h + 1) * DH], NBLK),
            "outmm": kmajor(np.ascontiguousarray(output[b].T).astype(f16), 4),
            "zmm": np.ascontiguousarray(input_z[b].T).astype(f16),
            "b1": np.ascontiguousarray(b1[dcols].reshape(NBLK, 128).T).astype(np.float32),
            "b2": np.ascontiguousarray(b2[dcols].reshape(NBLK, 128).T).astype(np.float32),
            "b3": np.ascontiguousarray(b3[dcols].reshape(NBLK, 128).T).astype(np.float32),
            "b4": np.ascontiguousarray(b4[dcols].reshape(NBLK, 128).T).astype(np.float32),
        })
    return in_maps


def _install_ntff_hook():
    """The agent image's antenv lacks axon_hooks; synthesize it so
    run_bass_kernel_spmd(trace=True) can collect NTFF profiles."""
    import types

    if "antenv.axon_hooks" in sys.modules:
        return
    import antenv

    mod = types.ModuleType("antenv.axon_hooks")
    holder = {"h": None}
    mod.set_axon_ntff_profile_hook = lambda h: holder.__setitem__("h", h)
    mod.get_axon_ntff_profile_hook = lambda: holder["h"]
    sys.modules["antenv.axon_hooks"] = mod
    antenv.axon_hooks = mod
    try:
        from trn_agent_boot.trn_boot import _ntff_profile_via_ctypes

        h = _ntff_profile_via_ctypes("/opt/axon/libaxon_pjrt.so")
        if h is not None:
            mod.set_axon_ntff_profile_hook(h)
    except Exception as e:
        print(f"ntff hook install failed: {e}", file=sys.stderr)


def _run(inputs, trace=False):
    from concourse.bass_utils import run_bass_kernel_spmd

    if trace:
        try:
            _install_ntff_hook()
        except Exception as e:
            print(f"ntff hook unavailable: {e}", file=sys.stderr)

    nc = _get_program()
    in_maps = _host_prep(**inputs)
    res = run_bass_kernel_spmd(
        nc, in_maps, core_ids=list(range(N_CORES)), trace=trace,
    )
    output = np.asarray(inputs["output"], dtype=np.float32)
    attn = np.empty((B, T, D), np.float32)
    slayout = os.environ.get("K_LAYOUT", "s") == "s"
    for c in range(N_CORES):
        b, h = divmod(c, 2)
        if slayout:
            attn[b, :, h * DH:(h + 1) * DH] = res.results[c]["attn_s"]
        else:
            at = res.results[c]["attn_t"]                 # [128, NBLK, T]
            at = at.transpose(1, 0, 2).reshape(DH, T)     # [d, t]
            attn[b, :, h * DH:(h + 1) * DH] = at.T
    concat = np.concatenate([output, attn], axis=-1)
    return (concat, attn), res


def kernel(**inputs):
    inputs = {k: np.asarray(v) for k, v in inputs.items()}
    (concat, attn), _ = _run(inputs, trace=False)
    return concat, attn
